# revision 7
# baseline (speedup 1.0000x reference)
"""Trainium2 kernel for nn_LinearMem: bit-sliced int8-quantized linear layer.

Math: the reference splits round(x/sx) and round(w.T/sw) into two's-complement
bit-planes (widths 1,1,2,4) and recombines 16 per-slice-pair matmuls with
2^shift weights.  That recombination is exactly sum_i 2^sh_i * plane_i == q,
so the whole einsum equals qx @ qw^T with qx = round(x/sx), qw = round(w/sw)
(clip to +-127 is a no-op since |x|/sx <= 127 by construction).  Every product
and partial sum is an integer < 2^24, so a bf16 x bf16 matmul with f32 PSUM
accumulation reproduces the reference bitwise (int8 values are exact in bf16).

Quantization itself needs an exact IEEE f32 divide to match the reference's
rounding; Trainium has no divide instruction on any engine, so the int8
quantization + shard layout prep is done host-side (as in real quantized
inference, where weights are quantized offline).  The device does all 17
GFLOP of matmul plus int8->bf16 expansion and dequantize + add bias.

Distribution (8 NeuronCores, tensor-parallel 2x4 grid):
  core c = (i, j): i = c//4 selects token rows (M/2 = 1024), j = c%4 selects
  out_features (N/4 = 512).

Schedule (per core), designed off the perfetto trace of the v0 kernel:
  - "head" DMA packs w-chunk0 (k-blocks 0-7) + x-tile0 per-partition
    contiguously (6 KiB/partition descriptors) so the 0.75 MiB needed for
    the first matmul group lands at near-line-rate; remaining x tiles and
    w-chunk1 stream behind it on the same HWDGE ring in consumption order.
  - PE warmup: gpsimd memset (earliest engine) + 10 dummy matmuls release
    the HAM clock gate (1.2 -> 2.4 GHz) while the head DMA lands.
  - k-phase-split matmul: phase 1 accumulates k-blocks 0-7 for all 8
    m-tiles (needs only w-chunk0), phase 2 adds k-blocks 8-15; the 8
    accumulators occupy all 8 PSUM banks across both phases, which lets
    w-chunk1 load last without stalling the PE.
  - int8->bf16 expansion split across DVE+ACT sized to their measured
    element rates; w-chunk1 expands on ACT slack during phase 1.
  - Dequant (acc*s + bias fused on DVE) + store per m-tile overlaps the
    next m-tile's matmuls; the last m-tile's phase 2 runs as two N-halves
    so its dequant+store overlaps the final matmuls, shortening the
    end-of-kernel DMA-receipt tail.
"""

import sys

if "/opt/trn_rl_repo" not in sys.path:
    sys.path.insert(0, "/opt/trn_rl_repo")

import ml_dtypes
import numpy as np

import concourse.bacc as bacc
import concourse.mybir as mybir
import concourse.tile as tile
from concourse.bass_utils import run_bass_kernel_spmd

M, K, N = 2048, 2048, 2048
PM, PN = 2, 4  # grid: M split PM ways, N split PN ways
MS, NS = M // PM, N // PN  # per-core shard sizes: 1024, 512

F32 = mybir.dt.float32
BF16 = mybir.dt.bfloat16
I8 = mybir.dt.int8

MT = MS // 128  # 8 m-tiles
KT = K // 128  # 16 k-blocks
WKB = KT // 2  # 8 k-blocks per w chunk
HEADB = WKB * NS + KT * 128  # 6144 bytes/partition: w-chunk0 + x-tile0


def _build_program():
    nc = bacc.Bacc("TRN2", target_bir_lowering=False, debug=False, num_devices=8)

    # head: per-partition [w0 (8 kb x 512), x0 (16 kb x 128)] contiguous
    head_in = nc.dram_tensor("head_sh", [128, HEADB], I8, kind="ExternalInput")
    # x m-tiles 1..7, each [128, KT, 128] in SBUF tile order
    qx_in = nc.dram_tensor("qxr_sh", [MT - 1, 128, KT, 128], I8, kind="ExternalInput")
    # w chunk 1 (k-blocks 8..15)
    qw1_in = nc.dram_tensor("qw1_sh", [128, WKB, NS], I8, kind="ExternalInput")
    b_in = nc.dram_tensor("b_sh", [1, NS], F32, kind="ExternalInput")
    scl_in = nc.dram_tensor("scl", [1, 4], F32, kind="ExternalInput")
    out_t = nc.dram_tensor("out_sh", [MS, NS], F32, kind="ExternalOutput")

    with tile.TileContext(nc) as tc:
        with (
            tc.tile_pool(name="const", bufs=1) as const,
            tc.tile_pool(name="i8", bufs=1) as i8p,
            tc.tile_pool(name="bf", bufs=1) as bfp,
            tc.tile_pool(name="out", bufs=3) as op,
            tc.tile_pool(name="psum", bufs=1, space="PSUM") as ps,
        ):
            # PE warmup source on gpsimd: it is the first engine to enter
            # main, so the warmup matmuls (and the HAM 3.4us window) start
            # as early as possible.  Nonzero data: zero MACs are power-gated.
            zsrc = const.tile([128, NS], BF16, tag="zsrc")
            nc.gpsimd.memset(zsrc[:], 1.0)
            # 8-deep "acc" ring = all 8 PSUM banks; zacc takes slot 0 and the
            # 8th accumulator (acc7) wraps onto it after warmup completes.
            zacc = ps.tile([128, NS], F32, tag="acc", bufs=8, name="zacc")
            for _ in range(10):
                nc.tensor.matmul(zacc[:], zsrc[:, 0:128], zsrc[:], start=True, stop=True)

            # input loads, one HWDGE ring, in consumption order; w-chunk1
            # last (not needed until phase 2)
            head = i8p.tile([128, HEADB], I8, tag="head", name="head")
            w8_0 = head[:, 0 : WKB * NS].rearrange("p (kb n) -> p kb n", kb=WKB)
            x8_0 = head[:, WKB * NS : HEADB].rearrange("p (kb m) -> p kb m", kb=KT)
            nc.sync.dma_start(head[:], head_in[:])
            x8 = [
                i8p.tile([128, KT, 128], I8, tag=f"x8_{m}", name=f"x8_{m}")
                for m in range(1, MT)
            ]
            for m in range(1, MT):
                nc.sync.dma_start(x8[m - 1][:], qx_in[m - 1])
            w8_1 = i8p.tile([128, WKB, NS], I8, tag="w8_1", name="w8_1")
            nc.sync.dma_start(w8_1[:], qw1_in[:])

            # constants via SWDGE (gpsimd)
            scl_row = const.tile([1, 4], F32, tag="scl_row")
            nc.gpsimd.dma_start(scl_row[:], scl_in[:])
            sclb = const.tile([128, 4], F32, tag="sclb")
            nc.gpsimd.partition_broadcast(sclb[:], scl_row[:], channels=128)
            s_ap = sclb[:, 0:1]  # dequant scale sx*sw

            bias_row = const.tile([1, NS], F32, tag="bias_row")
            nc.gpsimd.dma_start(bias_row[:], b_in[:])
            bias_b = const.tile([128, NS], F32, tag="bias_b")
            nc.gpsimd.partition_broadcast(bias_b[:], bias_row[:], channels=128)

            # int8 -> bf16 expansion.  DVE ~215 elem/ns, ACT ~118 elem/ns.
            wt = [
                bfp.tile([128, WKB, NS], BF16, tag=f"w{c}", name=f"w{c}")
                for c in range(2)
            ]
            xb = [
                bfp.tile([128, KT, 128], BF16, tag=f"x{m}", name=f"x{m}")
                for m in range(MT)
            ]

            # head expansion, fine-grained so the first matmuls can chase it:
            # DVE: w0 kb0-2, x0 kb0-9, w0 kb3-4;  ACT: w0 kb5-7, x0 kb10-15
            nc.vector.tensor_copy(wt[0][:, 0:3, :], w8_0[:, 0:3, :])
            nc.scalar.activation(
                wt[0][:, 5:WKB, :], w8_0[:, 5:WKB, :],
                mybir.ActivationFunctionType.Copy,
            )
            nc.vector.tensor_copy(xb[0][:, 0:10, :], x8_0[:, 0:10, :])
            nc.vector.tensor_copy(wt[0][:, 3:5, :], w8_0[:, 3:5, :])
            nc.scalar.activation(
                xb[0][:, 10:KT, :], x8_0[:, 10:KT, :],
                mybir.ActivationFunctionType.Copy,
            )

            accs = [
                ps.tile([128, NS], F32, tag="acc", bufs=8, name=f"acc{mb}")
                for mb in range(MT)
            ]

            # phase 1: k-blocks 0..7 for every m-tile (w-chunk0 only);
            # interleave the x-tile expansions and w-chunk1's ACT expansion
            # into the emission stream so each engine's FIFO stays in
            # just-in-time order.
            for mb in range(MT):
                for kb in range(WKB):
                    # mb7's group "closes" at kb7 purely for the simulator's
                    # group tracker (stop is a no-op on HW; has_written
                    # decides accumulate-vs-overwrite) so that its phase 2
                    # can run as two independently-closed N-halves.
                    nc.tensor.matmul(
                        accs[mb][:],
                        xb[mb][:, kb, :],
                        wt[0][:, kb, :],
                        start=(kb == 0),
                        stop=(mb == MT - 1 and kb == WKB - 1),
                    )
                nxt = mb + 1
                if nxt < MT:
                    # x tiles 1-3: DVE 10 / ACT 6; 4-7: DVE 12 / ACT 4
                    # (ACT slack in later windows expands w-chunk1)
                    h = 10 if nxt <= 3 else 12
                    nc.vector.tensor_copy(xb[nxt][:, 0:h, :], x8[nxt - 1][:, 0:h, :])
                    nc.scalar.activation(
                        xb[nxt][:, h:KT, :], x8[nxt - 1][:, h:KT, :],
                        mybir.ActivationFunctionType.Copy,
                    )
                if mb >= 3:
                    wb = 2 * (mb - 3)  # mb 3..6 -> w1 kb pairs (0,1)..(6,7)
                    if wb < WKB:
                        nc.scalar.activation(
                            wt[1][:, wb : wb + 2, :], w8_1[:, wb : wb + 2, :],
                            mybir.ActivationFunctionType.Copy,
                        )

            # phase 2: k-blocks 8..15; dequant+store each m-tile as its
            # accumulation closes.  Last m-tile runs as two N-halves so its
            # dequant+store overlaps the final matmuls.
            def dequant_store(acc_ap, rows, cols, eng):
                o2 = op.tile([128, NS], F32, tag="o2")
                nc.vector.scalar_tensor_tensor(
                    o2[:, cols], acc_ap, s_ap, bias_b[:, cols],
                    op0=mybir.AluOpType.mult, op1=mybir.AluOpType.add,
                )
                eng.dma_start(rows[:, cols], o2[:, cols])

            for mb in range(MT - 1):
                for kb in range(WKB):
                    nc.tensor.matmul(
                        accs[mb][:],
                        xb[mb][:, WKB + kb, :],
                        wt[1][:, kb, :],
                        start=False,
                        stop=(kb == WKB - 1),
                    )
                rows = out_t[mb * 128 : (mb + 1) * 128, :]
                eng = nc.sync if mb % 2 == 0 else nc.scalar
                dequant_store(accs[mb][:], rows, slice(0, NS), eng)

            mb = MT - 1
            rows = out_t[mb * 128 : (mb + 1) * 128, :]
            hn = NS // 2
            for h, eng in ((0, nc.sync), (1, nc.scalar)):
                cols = slice(h * hn, (h + 1) * hn)
                for kb in range(WKB):
                    nc.tensor.matmul(
                        accs[mb][:, cols],
                        xb[mb][:, WKB + kb, :],
                        wt[1][:, kb, cols],
                        start=False,
                        stop=(kb == WKB - 1),
                        skip_group_check=True,
                    )
                dequant_store(accs[mb][:, cols], rows, cols, eng)

    nc.compile()
    return nc


_NC = None


def _get_nc():
    global _NC
    if _NC is None:
        _NC = _build_program()
    return _NC


def _quantize(a):
    """Exactly the reference's quantization: scale = amax/127 (f32 IEEE),
    q = clip(round-half-even(a / scale), -127, 127)."""
    amax = np.float32(np.max(np.abs(a)))
    scale = amax / np.float32(127.0)
    q = np.clip(np.round((a / scale).astype(np.float32)), -127.0, 127.0)
    return q.astype(np.int8), scale


def kernel(x, weight, bias, _trace=False):
    x = np.asarray(x, dtype=np.float32)
    weight = np.asarray(weight, dtype=np.float32)
    bias = np.asarray(bias, dtype=np.float32)

    qx, sx = _quantize(x)
    qw, sw = _quantize(weight)
    s = sx * sw
    scl = np.array([[s, sx, sw, 0.0]], dtype=np.float32)

    qxt = qx.T  # [K, M]
    qwt = qw.T  # [K, N]

    in_maps = []
    for c in range(8):
        i, j = divmod(c, PN)
        xs = qxt[:, i * MS : (i + 1) * MS]  # [K, MS]
        # [MT, 128, KT, 128]: tile mb, partition k%128, block k//128, col m
        xs = np.ascontiguousarray(
            xs.reshape(KT, 128, MT, 128).transpose(2, 1, 0, 3)
        )
        ws = qwt[:, j * NS : (j + 1) * NS]  # [K, NS]
        # [2, 128, WKB, NS]: chunk, partition k%128, block (k//128)%8, col n
        ws = np.ascontiguousarray(
            ws.reshape(2, WKB, 128, NS).transpose(0, 2, 1, 3)
        )
        head = np.concatenate(
            [ws[0].reshape(128, WKB * NS), xs[0].reshape(128, KT * 128)], axis=1
        )
        in_maps.append(
            {
                "head_sh": np.ascontiguousarray(head),
                "qxr_sh": xs[1:],
                "qw1_sh": ws[1],
                "b_sh": bias[j * NS : (j + 1) * NS].reshape(1, NS),
                "scl": scl,
            }
        )

    nc = _get_nc()
    try:
        res = run_bass_kernel_spmd(nc, in_maps, core_ids=list(range(8)), trace=_trace)
    except Exception:
        # rare transient NRT device hiccups recover on retry
        res = run_bass_kernel_spmd(nc, in_maps, core_ids=list(range(8)), trace=_trace)

    out = np.empty((M, N), np.float32)
    for c in range(8):
        i, j = divmod(c, PN)
        out[i * MS : (i + 1) * MS, j * NS : (j + 1) * NS] = res.results[c]["out_sh"]
    if _trace:
        return out, res
    return out


# revision 8
# speedup vs baseline: 1.0467x; 1.0467x over previous
"""Trainium2 kernel for nn_LinearMem: bit-sliced int8-quantized linear layer.

Math: the reference splits round(x/sx) and round(w.T/sw) into two's-complement
bit-planes (widths 1,1,2,4) and recombines 16 per-slice-pair matmuls with
2^shift weights.  That recombination is exactly sum_i 2^sh_i * plane_i == q,
so the whole einsum equals qx @ qw^T with qx = round(x/sx), qw = round(w/sw)
(clip to +-127 is a no-op since |x|/sx <= 127 by construction).  Every product
and partial sum is an integer < 2^24, so a bf16 x bf16 matmul with f32 PSUM
accumulation reproduces the reference bitwise (int8 values are exact in bf16).

Quantization itself needs an exact IEEE f32 divide to match the reference's
rounding; Trainium has no divide instruction on any engine, so the int8
quantization + shard layout prep is done host-side (as in real quantized
inference, where weights are quantized offline).  The device does all 17
GFLOP of matmul plus int8->bf16 expansion and dequantize + add bias.

Distribution (8 NeuronCores, tensor-parallel 2x4 grid):
  core c = (i, j): i = c//4 selects token rows (M/2 = 1024), j = c%4 selects
  out_features (N/4 = 512).

Schedule (per core), tuned against perfetto traces:
  - head DMAs pack w-chunk0 k-blocks 0-3 + x-tile0 (then w0 k-blocks 4-7)
    per-partition contiguously so the first matmul group's data lands with
    large descriptors at near-line-rate; remaining x tiles and w-chunk1
    stream behind on the same HWDGE ring in consumption order.
  - PE warmup: 8 N=512 + 8 N=128 dummy matmuls bridge the HAM clock-gate
    window (1.2 -> 2.4 GHz) and run CONTINUOUSLY into the real stream; any
    PE idle gap here lets HAM re-throttle and costs ~1.7us of half-rate
    matmuls (observed).
  - k-phase-split matmul: phase 1 accumulates k-blocks 0-7 for all 8
    m-tiles (needs only w-chunk0), phase 2 adds k-blocks 8-15; the 8
    accumulators occupy all 8 PSUM banks across both phases, which lets
    w-chunk1 load last without stalling the PE.
  - int8->bf16 expansion interleaved DVE/ACT in consumption order;
    w-chunk1 expands on ACT slack during phase 1.
  - Dequant (acc*s + bias fused on DVE) + store per m-tile overlaps the
    next m-tile's matmuls; the final m-tile's store is quartered across
    both HWDGE rings so the end-of-kernel write-receipt tail is short.
"""

import sys

if "/opt/trn_rl_repo" not in sys.path:
    sys.path.insert(0, "/opt/trn_rl_repo")

import ml_dtypes
import numpy as np

import concourse.bacc as bacc
import concourse.mybir as mybir
import concourse.tile as tile
from concourse.bass_utils import run_bass_kernel_spmd

M, K, N = 2048, 2048, 2048
PM, PN = 2, 4  # grid: M split PM ways, N split PN ways
MS, NS = M // PM, N // PN  # per-core shard sizes: 1024, 512

F32 = mybir.dt.float32
BF16 = mybir.dt.bfloat16
I8 = mybir.dt.int8

MT = MS // 128  # 8 m-tiles
KT = K // 128  # 16 k-blocks
WKB = KT // 2  # 8 k-blocks per w chunk
H1B = 4 * NS + KT * 128  # head1: w0 kb0-3 + x0  (4096 B/partition)
H2B = 4 * NS  # head2: w0 kb4-7 (2048 B/partition)


def _build_program():
    nc = bacc.Bacc("TRN2", target_bir_lowering=False, debug=False, num_devices=8)

    head1_in = nc.dram_tensor("head1_sh", [128, H1B], I8, kind="ExternalInput")
    head2_in = nc.dram_tensor("head2_sh", [128, H2B], I8, kind="ExternalInput")
    qx_in = nc.dram_tensor("qxr_sh", [MT - 1, 128, KT, 128], I8, kind="ExternalInput")
    qw1_in = nc.dram_tensor("qw1_sh", [128, WKB, NS], I8, kind="ExternalInput")
    b_in = nc.dram_tensor("b_sh", [1, NS], F32, kind="ExternalInput")
    scl_in = nc.dram_tensor("scl", [1, 4], F32, kind="ExternalInput")
    out_t = nc.dram_tensor("out_sh", [MS, NS], F32, kind="ExternalOutput")

    with tile.TileContext(nc) as tc:
        with (
            tc.tile_pool(name="const", bufs=1) as const,
            tc.tile_pool(name="i8", bufs=1) as i8p,
            tc.tile_pool(name="bf", bufs=1) as bfp,
            tc.tile_pool(name="out", bufs=3) as op,
            tc.tile_pool(name="psum", bufs=1, space="PSUM") as ps,
        ):
            # PE warmup source on gpsimd (first engine into main).
            # Nonzero data: zero MACs are power-gated and don't warm HAM.
            zsrc = const.tile([128, NS], BF16, tag="zsrc")
            nc.gpsimd.memset(zsrc[:], 1.0)
            # 8-deep "acc" ring = all 8 PSUM banks; zacc takes slot 0 and
            # acc7 wraps onto it after warmup completes.
            zacc = ps.tile([128, NS], F32, tag="acc", bufs=8, name="zacc")
            for _ in range(8):
                nc.tensor.matmul(zacc[:], zsrc[:, 0:128], zsrc[:], start=True, stop=True)
            for _ in range(8):
                nc.tensor.matmul(
                    zacc[:, 0:128], zsrc[:, 0:128], zsrc[:, 0:128],
                    start=True, stop=True,
                )

            # input loads, one HWDGE ring, in consumption order
            h1 = i8p.tile([128, H1B], I8, tag="h1", name="h1")
            w0a = h1[:, 0 : 4 * NS].rearrange("p (kb n) -> p kb n", kb=4)
            x8_0 = h1[:, 4 * NS : H1B].rearrange("p (kb m) -> p kb m", kb=KT)
            nc.sync.dma_start(h1[:], head1_in[:])
            h2 = i8p.tile([128, H2B], I8, tag="h2", name="h2")
            w0b = h2[:].rearrange("p (kb n) -> p kb n", kb=4)
            nc.sync.dma_start(h2[:], head2_in[:])
            x8 = [
                i8p.tile([128, KT, 128], I8, tag=f"x8_{m}", name=f"x8_{m}")
                for m in range(1, MT)
            ]
            for m in range(1, MT):
                nc.sync.dma_start(x8[m - 1][:], qx_in[m - 1])
            w8_1 = i8p.tile([128, WKB, NS], I8, tag="w8_1", name="w8_1")
            nc.sync.dma_start(w8_1[:], qw1_in[:])

            # constants via SWDGE (gpsimd)
            scl_row = const.tile([1, 4], F32, tag="scl_row")
            nc.gpsimd.dma_start(scl_row[:], scl_in[:])
            sclb = const.tile([128, 4], F32, tag="sclb")
            nc.gpsimd.partition_broadcast(sclb[:], scl_row[:], channels=128)
            s_ap = sclb[:, 0:1]  # dequant scale sx*sw

            bias_row = const.tile([1, NS], F32, tag="bias_row")
            nc.gpsimd.dma_start(bias_row[:], b_in[:])
            bias_b = const.tile([128, NS], F32, tag="bias_b")
            nc.gpsimd.partition_broadcast(bias_b[:], bias_row[:], channels=128)

            # int8 -> bf16 expansion targets
            wt0 = bfp.tile([128, WKB, NS], BF16, tag="w0", name="wt0")
            wt1 = bfp.tile([128, WKB, NS], BF16, tag="w1", name="wt1")
            xb = [
                bfp.tile([128, KT, 128], BF16, tag=f"x{m}", name=f"x{m}")
                for m in range(MT)
            ]

            # head expansion, fine-grained in consumption order so the first
            # matmuls chase the casts.  DVE ~215 elem/ns, ACT ~118 elem/ns.
            cp = mybir.ActivationFunctionType.Copy
            nc.vector.tensor_copy(wt0[:, 0:2, :], w0a[:, 0:2, :])
            nc.vector.tensor_copy(xb[0][:, 0:2, :], x8_0[:, 0:2, :])
            nc.scalar.activation(xb[0][:, 10:KT, :], x8_0[:, 10:KT, :], cp)
            nc.vector.tensor_copy(xb[0][:, 2:10, :], x8_0[:, 2:10, :])
            nc.vector.tensor_copy(wt0[:, 2:4, :], w0a[:, 2:4, :])
            nc.vector.tensor_copy(wt0[:, 4:6, :], w0b[:, 0:2, :])
            nc.scalar.activation(wt0[:, 6:WKB, :], w0b[:, 2:4, :], cp)

            accs = [
                ps.tile([128, NS], F32, tag="acc", bufs=8, name=f"acc{mb}")
                for mb in range(MT)
            ]

            # phase 1: k-blocks 0..7 for every m-tile (w-chunk0 only), with
            # x-tile expansions and w-chunk1's ACT expansion interleaved in
            # just-in-time order.
            for mb in range(MT):
                for kb in range(WKB):
                    nc.tensor.matmul(
                        accs[mb][:],
                        xb[mb][:, kb, :],
                        wt0[:, kb, :],
                        start=(kb == 0),
                        stop=False,
                    )
                nxt = mb + 1
                if nxt < MT:
                    # x tiles 1-3: DVE 10 / ACT 6; 4-7: DVE 12 / ACT 4
                    # (the freed ACT slack expands w-chunk1)
                    h = 10 if nxt <= 3 else 12
                    nc.vector.tensor_copy(xb[nxt][:, 0:h, :], x8[nxt - 1][:, 0:h, :])
                    nc.scalar.activation(
                        xb[nxt][:, h:KT, :], x8[nxt - 1][:, h:KT, :], cp
                    )
                if mb >= 3:
                    wb = 2 * (mb - 3)  # mb 3..6 -> w1 kb pairs (0,1)..(6,7)
                    if wb < WKB:
                        nc.scalar.activation(
                            wt1[:, wb : wb + 2, :], w8_1[:, wb : wb + 2, :], cp
                        )

            # phase 2: k-blocks 8..15; dequant+store each m-tile as its
            # accumulation closes.  Final m-tile's store is quartered across
            # both rings to shorten the exposed write-receipt tail.
            for mb in range(MT):
                for kb in range(WKB):
                    nc.tensor.matmul(
                        accs[mb][:],
                        xb[mb][:, WKB + kb, :],
                        wt1[:, kb, :],
                        start=False,
                        stop=(kb == WKB - 1),
                    )
                rows = out_t[mb * 128 : (mb + 1) * 128, :]
                o2 = op.tile([128, NS], F32, tag="o2")
                if mb < MT - 1:
                    nc.vector.scalar_tensor_tensor(
                        o2[:], accs[mb][:], s_ap, bias_b[:],
                        op0=mybir.AluOpType.mult, op1=mybir.AluOpType.add,
                    )
                    eng = nc.sync if mb % 2 == 0 else nc.scalar
                    eng.dma_start(rows, o2[:])
                else:
                    qn = NS // 4
                    for q in range(4):
                        cols = slice(q * qn, (q + 1) * qn)
                        nc.vector.scalar_tensor_tensor(
                            o2[:, cols], accs[mb][:, cols], s_ap, bias_b[:, cols],
                            op0=mybir.AluOpType.mult, op1=mybir.AluOpType.add,
                        )
                        eng = nc.sync if q % 2 == 0 else nc.scalar
                        eng.dma_start(rows[:, cols], o2[:, cols])

    nc.compile()
    return nc


_NC = None


def _get_nc():
    global _NC
    if _NC is None:
        _NC = _build_program()
    return _NC


def _quantize(a):
    """Exactly the reference's quantization: scale = amax/127 (f32 IEEE),
    q = clip(round-half-even(a / scale), -127, 127)."""
    amax = np.float32(np.max(np.abs(a)))
    scale = amax / np.float32(127.0)
    q = np.clip(np.round((a / scale).astype(np.float32)), -127.0, 127.0)
    return q.astype(np.int8), scale


def _shard_inputs(qx, qw, bias, scl):
    qxt = qx.T  # [K, M]
    qwt = qw.T  # [K, N]
    in_maps = []
    for c in range(8):
        i, j = divmod(c, PN)
        xs = qxt[:, i * MS : (i + 1) * MS]  # [K, MS]
        # [MT, 128, KT, 128]: tile mb, partition k%128, block k//128, col m
        xs = np.ascontiguousarray(xs.reshape(KT, 128, MT, 128).transpose(2, 1, 0, 3))
        ws = qwt[:, j * NS : (j + 1) * NS]  # [K, NS]
        # [KT, 128, NS] -> per k-block
        ws = np.ascontiguousarray(ws.reshape(KT, 128, NS).transpose(1, 0, 2))
        # ws is now [128, KT, NS]
        head1 = np.concatenate(
            [ws[:, 0:4].reshape(128, 4 * NS), xs[0].reshape(128, KT * 128)], axis=1
        )
        in_maps.append(
            {
                "head1_sh": np.ascontiguousarray(head1),
                "head2_sh": np.ascontiguousarray(ws[:, 4:8].reshape(128, 4 * NS)),
                "qxr_sh": xs[1:],
                "qw1_sh": np.ascontiguousarray(ws[:, 8:16]),
                "b_sh": bias[j * NS : (j + 1) * NS].reshape(1, NS),
                "scl": scl,
            }
        )
    return in_maps


def kernel(x, weight, bias, _trace=False):
    x = np.asarray(x, dtype=np.float32)
    weight = np.asarray(weight, dtype=np.float32)
    bias = np.asarray(bias, dtype=np.float32)

    qx, sx = _quantize(x)
    qw, sw = _quantize(weight)
    s = sx * sw
    scl = np.array([[s, sx, sw, 0.0]], dtype=np.float32)

    in_maps = _shard_inputs(qx, qw, bias, scl)

    nc = _get_nc()
    try:
        res = run_bass_kernel_spmd(nc, in_maps, core_ids=list(range(8)), trace=_trace)
    except Exception:
        # rare transient NRT device hiccups recover on retry
        res = run_bass_kernel_spmd(nc, in_maps, core_ids=list(range(8)), trace=_trace)

    out = np.empty((M, N), np.float32)
    for c in range(8):
        i, j = divmod(c, PN)
        out[i * MS : (i + 1) * MS, j * NS : (j + 1) * NS] = res.results[c]["out_sh"]
    if _trace:
        return out, res
    return out
